# revision 7
# baseline (speedup 1.0000x reference)
"""LocallyConnected1d Trainium2 kernel.

out[b, o, l] = sum_{c,k} x[b, c, l+k] * weight[o, c, l, k] + bias[o, l]
  x: (32, 128, 2050) f32, weight: (128, 128, 2048, 3) f32, bias: (128, 2048) f32
  out: (32, 128, 2048) f32

Sharding: sequence-parallel over L across 8 cores (each core owns 256 output
positions, its private 50.3 MB weight slice, a 258-wide x window and a bias
slice).  Weight streaming from HBM (~50 MB/core, used exactly once) is the
roofline; everything else hides under it.

Per-core kernel: per output position l, a (Cout x Cin*K x B) GEMM with
position-unique weights.  Weights are DMA'd as (c_part, o, l, k) tiles
(768 B contiguous descriptors), and each (l, k) pair is one accumulating
PE matmul: lhsT = W[o_half, :, l, k]^T (K=128, M=64), rhs = x[:, :, l+k]^T
(K=128, N=32) -> PSUM (64, 16l x 32b).  DVE adds bias (broadcast AP) while
transposing PSUM (l, b) -> staging (b, l); staging DMAs out with 512 B runs.
"""

import sys
import types

import numpy as np

import concourse.bass as bass
import concourse.mybir as mybir
import concourse.tile as tile
from concourse.vector_clock import ScopedClock, VectorClock
from concourse.bass_utils import run_bass_kernel_spmd

# ---------------------------------------------------------------------------
# Environment patches
# ---------------------------------------------------------------------------

# The walrus build in this image rejects instructions with >2 sem waits; the
# Tile tail drain carries one wait per logical processor.  Split them into
# single-wait nops on SP before the drain.
def _patched_drain_and_barrier(self, tick_clock, wait_clock):
    gc = tick_clock.global_clock
    n = len(gc)
    for proc in range(n):
        t = gc[proc]
        if t <= 0:
            continue
        single = VectorClock([0] * n)
        single.require_at_least(proc, t)
        inst = self.nc.sync.nop(hint="tail_drain_wait")
        wait_clock.add_sem_waits(inst.ins, ScopedClock({None: single}))
    self.nc.sync.drain()
    self.nc.all_engine_barrier()
    assert self.sems is not None
    popped = self.nc._tile_sem_poison_stack.pop()
    assert popped is self._sem_poison
    self.nc.clear_and_free_semaphores(list(self.sems.allocated().values()))
    self.nc.all_engine_barrier()


if not getattr(tile.TileContext, "_drain_patch_applied", False):
    tile.TileContext._drain_and_barrier = _patched_drain_and_barrier
    tile.TileContext._drain_patch_applied = True


def _split_multi_waits(nc: bass.Bass) -> int:
    """The hardware takes one sem wait per instruction; this walrus build
    rejects multi-wait instructions instead of splitting them.  Hoist all but
    the last wait of any multi-wait instruction onto single-wait nops inserted
    just before it in its engine's program order."""
    n_split = 0
    for f in nc.m.functions:
        for bb in f.blocks:
            insts = list(bb.instructions)
            out = []
            for inst in insts:
                si = inst.sync_info
                if si is not None and len(si.on_wait) > 1:
                    waits = list(si.on_wait)
                    for w in waits[:-1]:
                        nop = mybir.InstNoOp(
                            name=nc.get_next_instruction_name(),
                            engine=inst.engine,
                            ins=[],
                            outs=[],
                            sync_info=mybir.SyncInfo(on_wait=[w], on_update=[]),
                        )
                        out.append(nop)
                    si.on_wait = [waits[-1]]
                    n_split += 1
                out.append(inst)
            bb.instructions = out
    return n_split

# ---------------------------------------------------------------------------
# Problem constants (hardcoded from the module spec)
# ---------------------------------------------------------------------------
N_CORES = 8
B = 32
CIN = 128
COUT = 128
L = 2048
KS = 3
W_FULL = 2050

LSH = L // N_CORES          # 256 output positions per core
WW = LSH + KS - 1           # 258-wide x window per core

OC = 2                      # Cout split: 2 halves of 64
OH = COUT // OC             # 64
LC = 4                      # l-chunks per core
LT = LSH // LC              # 64 positions per weight tile
GROUP = 16                  # l-positions per PSUM tile (16*32 = 512 fp32 = 1 bank)
NG = LT // GROUP            # 4 PSUM groups per unit

F32 = mybir.dt.float32


def _build_nc(split: bool = True) -> bass.Bass:
    nc = bass.Bass()

    x_d = nc.declare_dram_parameter("x", [B, CIN, WW], F32, isOutput=False)
    w_d = nc.declare_dram_parameter("w", [COUT, CIN, LSH, KS], F32, isOutput=False)
    b_d = nc.declare_dram_parameter("b", [COUT, LSH], F32, isOutput=False)
    out_d = nc.declare_dram_parameter("out", [B, COUT, LSH], F32, isOutput=True)

    with tile.TileContext(nc) as tc:
        with (
            tc.tile_pool(name="xp", bufs=1) as xp,
            tc.tile_pool(name="bp", bufs=1) as bp,
            tc.tile_pool(name="wp", bufs=2) as wp,
            tc.tile_pool(name="sp", bufs=2) as sp,
            tc.tile_pool(name="pp", bufs=4, space="PSUM") as pp,
        ):
            # Persistent x: (c, b, w) so rhs for (l, k) is a stride-258 AP
            # over b.  DMA runs are 258*4 = 1032 B.
            x_sb = xp.tile([CIN, B, WW], F32)
            nc.sync.dma_start(x_sb[:], x_d.rearrange("b c w -> c b w"))

            # Bias, one tile per Cout-half so partitions line up with PSUM.
            bias_sb = []
            for oc in range(OC):
                bt = bp.tile([OH, LSH], F32, tag=f"bias{oc}", name=f"bias{oc}")
                nc.sync.dma_start(bt[:], b_d[oc * OH:(oc + 1) * OH, :])
                bias_sb.append(bt)

            for half in range(2):
                # staging tiles for this l-half: (o_half, b, 128 l)
                stages = []
                for oc in range(OC):
                    stages.append(
                        sp.tile([OH, B, 2 * LT], F32, tag="st",
                                name=f"st_h{half}_o{oc}")
                    )

                for lc2 in range(2):
                    lc = half * 2 + lc2
                    for oc in range(OC):
                        # weight tile (c, o', l'*k); DRAM runs 768 B
                        w_t = wp.tile([CIN, OH, LT * KS], F32, tag="w")
                        src = w_d[oc * OH:(oc + 1) * OH, :,
                                  lc * LT:(lc + 1) * LT, :]
                        nc.sync.dma_start(
                            w_t[:], src.rearrange("o c l k -> c o (l k)")
                        )

                        st = stages[oc]
                        for g in range(NG):
                            ps = pp.tile([OH, GROUP, B], F32, tag="ps")
                            for li in range(GROUP):
                                l_loc = g * GROUP + li
                                l_sh = lc * LT + l_loc
                                for k in range(KS):
                                    nc.tensor.matmul(
                                        ps[:, li, :],
                                        w_t[:, :, l_loc * KS + k],
                                        x_sb[:, :, l_sh + k],
                                        start=(k == 0),
                                        stop=(k == KS - 1),
                                    )
                            # PSUM (o', l, b) + bias -> staging (o', b, l)
                            l0 = lc * LT + g * GROUP
                            sl0 = lc2 * LT + g * GROUP
                            nc.vector.tensor_tensor(
                                st[:, :, sl0:sl0 + GROUP],
                                ps.transpose([0, 2, 1]),
                                bias_sb[oc][:, None, l0:l0 + GROUP]
                                .to_broadcast((OH, B, GROUP)),
                                mybir.AluOpType.add,
                            )

                # flush both staging tiles for this half (512 B runs)
                for oc in range(OC):
                    dst = out_d[:, oc * OH:(oc + 1) * OH,
                                half * 2 * LT:(half + 1) * 2 * LT]
                    nc.scalar.dma_start(
                        dst.rearrange("b o l -> o b l"), stages[oc][:]
                    )

    if split:
        _split_multi_waits(nc)
    return nc


_NC_CACHE = None


def _get_nc() -> bass.Bass:
    global _NC_CACHE
    if _NC_CACHE is None:
        _NC_CACHE = _build_nc()
    return _NC_CACHE


def shard_inputs(x, weight, bias):
    x = np.asarray(x, dtype=np.float32)
    weight = np.asarray(weight, dtype=np.float32)
    bias = np.asarray(bias, dtype=np.float32)
    in_maps = []
    for i in range(N_CORES):
        l0 = i * LSH
        in_maps.append({
            "x": np.ascontiguousarray(x[:, :, l0:l0 + WW]),
            "w": np.ascontiguousarray(weight[:, :, l0:l0 + LSH, :]),
            "b": np.ascontiguousarray(bias[:, l0:l0 + LSH]),
        })
    return in_maps


def gather_output(results):
    out = np.empty((B, COUT, L), dtype=np.float32)
    for i in range(N_CORES):
        out[:, :, i * LSH:(i + 1) * LSH] = results[i]["out"]
    return out


def kernel(x, weight, bias):
    nc = _get_nc()
    in_maps = shard_inputs(x, weight, bias)
    res = run_bass_kernel_spmd(nc, in_maps, core_ids=list(range(N_CORES)),
                               trace=False)
    return gather_output(res.results)


# revision 10
# speedup vs baseline: 1.6914x; 1.6914x over previous
"""LocallyConnected1d Trainium2 kernel (v2: x-stationary, fp32r, fused taps).

out[b, o, l] = sum_{c,k} x[b, c, l+k] * weight[o, c, l, k] + bias[o, l]
  x: (32, 128, 2050) f32, weight: (128, 128, 2048, 3) f32, bias: (128, 2048) f32
  out: (32, 128, 2048) f32

Sharding: sequence-parallel over L across 8 cores (each core owns 256 output
positions, its private 50.3 MB weight slice, a 258-wide x window, and a
transposed bias slice).  Weight streaming from HBM is the roofline.

Per-core compute: out.T[b, l, o] = sum_{c} x[b, c, m] * W[o, c, l, m-l] for
each x column m.  The x column is the PE stationary operand (lhsT, K=128 c,
M=32 b, loaded once per column); the weights are the moving operand.  For one
column m, the contributions to l = m-2..m form an anti-diagonal in the
weight tile's (l, k) plane — stride 2 in the flattened l*3+k axis — so all
three taps fuse into ONE matmul with N = 3*128 = 384 moving columns.  With
float32r (single-pass fp32) and N >= 256 the PE streams 1 row/cycle.

PSUM: one bank holds out.T slice (32 b, 4 l, 128 o).  Each bank takes 7
matmuls: a K=1 ones x biasT matmul (start=True: clears the bank, seeds the
bias, sets every has_written bit) then 6 weight matmuls (m = 4j .. 4j+5,
clipped to the bank; per-element has_written makes them pure accumulates).
DVE then copies the bank to an (o, l)-ordered staging tile for the out DMA.
"""

import numpy as np

import concourse.bass as bass
import concourse.mybir as mybir
import concourse.tile as tile
from concourse.vector_clock import ScopedClock, VectorClock
from concourse.bass_utils import run_bass_kernel_spmd

# ---------------------------------------------------------------------------
# Environment patches
# ---------------------------------------------------------------------------

# The walrus build in this image rejects instructions with >1 sem wait; the
# Tile tail drain carries one wait per logical processor.  Split them into
# single-wait nops on SP before the drain.
def _patched_drain_and_barrier(self, tick_clock, wait_clock):
    gc = tick_clock.global_clock
    n = len(gc)
    for proc in range(n):
        t = gc[proc]
        if t <= 0:
            continue
        single = VectorClock([0] * n)
        single.require_at_least(proc, t)
        inst = self.nc.sync.nop(hint="tail_drain_wait")
        wait_clock.add_sem_waits(inst.ins, ScopedClock({None: single}))
    self.nc.sync.drain()
    self.nc.all_engine_barrier()
    assert self.sems is not None
    popped = self.nc._tile_sem_poison_stack.pop()
    assert popped is self._sem_poison
    self.nc.clear_and_free_semaphores(list(self.sems.allocated().values()))
    self.nc.all_engine_barrier()


if not getattr(tile.TileContext, "_drain_patch_applied", False):
    tile.TileContext._drain_and_barrier = _patched_drain_and_barrier
    tile.TileContext._drain_patch_applied = True


def _split_multi_waits(nc: bass.Bass) -> int:
    """Hoist all but the last wait of any multi-wait instruction onto
    single-wait nops inserted just before it in its engine's program order
    (the hardware takes one sem wait per instruction; this walrus build
    rejects multi-wait instructions instead of splitting them)."""
    n_split = 0
    for f in nc.m.functions:
        for bb in f.blocks:
            insts = list(bb.instructions)
            out = []
            for inst in insts:
                si = inst.sync_info
                if si is not None and len(si.on_wait) > 1:
                    waits = list(si.on_wait)
                    for w in waits[:-1]:
                        nop = mybir.InstNoOp(
                            name=nc.get_next_instruction_name(),
                            engine=inst.engine,
                            ins=[],
                            outs=[],
                            sync_info=mybir.SyncInfo(on_wait=[w], on_update=[]),
                        )
                        out.append(nop)
                    si.on_wait = [waits[-1]]
                    n_split += 1
                out.append(inst)
            bb.instructions = out
    return n_split

# ---------------------------------------------------------------------------
# Problem constants (hardcoded from the module spec)
# ---------------------------------------------------------------------------
N_CORES = 8
B = 32
CIN = 128
COUT = 128
L = 2048
KS = 3
W_FULL = 2050

LSH = L // N_CORES          # 256 output positions per core
WW = LSH + KS - 1           # 258-wide x window per core

LT = 32                     # l positions per weight tile / staging window
NWIN = LSH // LT            # 8 windows per core
BANKL = 4                   # l positions per PSUM bank (4*128 = 512 fp32)
NBANK = LT // BANKL         # 8 banks per window
BCH = 16                    # l rows per biasT SBUF chunk

F32 = mybir.dt.float32
F32R = mybir.dt.float32r


def _build_nc(split: bool = True) -> bass.Bass:
    nc = bass.Bass()

    x_d = nc.declare_dram_parameter("x", [B, CIN, WW], F32R, isOutput=False)
    w_d = nc.declare_dram_parameter("w", [COUT, CIN, LSH, KS], F32R, isOutput=False)
    bt_d = nc.declare_dram_parameter("biasT", [LSH, COUT], F32R, isOutput=False)
    ones_d = nc.declare_dram_parameter("ones", [1, B], F32R, isOutput=False)
    out_d = nc.declare_dram_parameter("out", [B, COUT, LSH], F32, isOutput=True)

    with tile.TileContext(nc) as tc:
        with (
            tc.tile_pool(name="xp", bufs=1) as xp,
            tc.tile_pool(name="cp", bufs=1) as cp,
            tc.tile_pool(name="wp", bufs=2) as wp,
            tc.tile_pool(name="bp", bufs=2) as bp,
            tc.tile_pool(name="sp", bufs=2) as sp,
            tc.tile_pool(name="pp", bufs=8, space="PSUM") as pp,
        ):
            # Persistent x in (c, b, w) layout: the stationary operand for
            # column m is x_sb[:, :, m] (K=128 c, M=32 b).  DMA runs 1032 B.
            x_sb = xp.tile([CIN, B, WW], F32R)
            nc.sync.dma_start(x_sb[:], x_d.rearrange("b c w -> c b w"))

            ones = cp.tile([1, B], F32R)
            nc.sync.dma_start(ones[:], ones_d[:])

            for lc in range(NWIN):
                # weight tile (c, o, l*k); DRAM runs LT*KS*4 = 384 B
                w_t = wp.tile([CIN, COUT, LT * KS], F32R, tag="w", name="w_t")
                nc.sync.dma_start(
                    w_t[:],
                    w_d[:, :, lc * LT:(lc + 1) * LT, :]
                    .rearrange("o c l k -> c o (l k)"),
                )

                # biasT rows for this window, flattened on partition 0
                bias_ts = []
                for h in range(LT // BCH):
                    btile = bp.tile([1, BCH * COUT], F32R, tag="bt",
                                    name=f"bt_{lc}_{h}")
                    l0 = lc * LT + h * BCH
                    nc.sync.dma_start(
                        btile[:],
                        bt_d[l0:l0 + BCH, :].rearrange("l o -> (l o)")[None, :],
                    )
                    bias_ts.append(btile)

                st = sp.tile([B, COUT, LT], F32, tag="st", name=f"st_{lc}")

                for jb in range(NBANK):
                    ps = pp.tile([B, BANKL, COUT], F32, tag="ps", name="ps")
                    lw0 = jb * BANKL              # window-local l of bank start

                    # bias init: out[b, (l, o)] = 1[b] * biasT[(l, o)];
                    # start=True clears the bank and sets has_written.
                    bchunk = bias_ts[lw0 // BCH]
                    boff = (lw0 % BCH) * COUT
                    nc.tensor.matmul(
                        ps[:].rearrange("b l o -> b (l o)"),
                        ones[:],
                        bchunk[0:1, boff:boff + BANKL * COUT],
                        start=True,
                        stop=False,
                        skip_group_check=True,
                    )

                    # six weight matmuls: x columns m = bank start .. +5
                    for d in range(BANKL + KS - 1):
                        mw = lw0 + d                  # window-local x column
                        m = lc * LT + mw              # shard-local x column
                        lo = max(lw0, mw - (KS - 1))  # window-local l' range
                        hi = min(lw0 + BANKL - 1, mw)
                        nl = hi - lo + 1
                        # anti-diagonal AP over the weight tile: element
                        # (o, l', k=mw-l') at o*(LT*KS) + l'*3 + (mw-l')
                        # -> l' step 2, o step LT*KS
                        rhs = bass.AP(
                            w_t[:].tensor,
                            lo * KS + (mw - lo),
                            [[COUT * LT * KS, CIN], [2, nl], [LT * KS, COUT]],
                        )
                        nc.tensor.matmul(
                            ps[:, lo - lw0:hi - lw0 + 1, :],
                            x_sb[:, :, m],
                            rhs,
                            start=False,
                            stop=(d == BANKL + KS - 2),
                            skip_group_check=True,
                        )

                    # PSUM (b, l, o) -> staging (b, o, l)
                    nc.vector.tensor_copy(
                        st[:, :, lw0:lw0 + BANKL],
                        ps[:].transpose([0, 2, 1]),
                    )

                nc.scalar.dma_start(out_d[:, :, lc * LT:(lc + 1) * LT], st[:])

    if split:
        _split_multi_waits(nc)
    return nc


_NC_CACHE = None


def _get_nc() -> bass.Bass:
    global _NC_CACHE
    if _NC_CACHE is None:
        _NC_CACHE = _build_nc()
    return _NC_CACHE


def shard_inputs(x, weight, bias):
    x = np.asarray(x, dtype=np.float32)
    weight = np.asarray(weight, dtype=np.float32)
    bias = np.asarray(bias, dtype=np.float32)
    in_maps = []
    for i in range(N_CORES):
        l0 = i * LSH
        in_maps.append({
            "x": np.ascontiguousarray(x[:, :, l0:l0 + WW]),
            "w": np.ascontiguousarray(weight[:, :, l0:l0 + LSH, :]),
            "biasT": np.ascontiguousarray(bias[:, l0:l0 + LSH].T),
            "ones": np.ones((1, B), dtype=np.float32),
        })
    return in_maps


def gather_output(results):
    out = np.empty((B, COUT, L), dtype=np.float32)
    for i in range(N_CORES):
        out[:, :, i * LSH:(i + 1) * LSH] = results[i]["out"]
    return out


def kernel(x, weight, bias):
    nc = _get_nc()
    in_maps = shard_inputs(x, weight, bias)
    res = run_bass_kernel_spmd(nc, in_maps, core_ids=list(range(N_CORES)),
                               trace=False)
    return gather_output(res.results)


# revision 11
# speedup vs baseline: 1.7738x; 1.0487x over previous
"""LocallyConnected1d Trainium2 kernel (v2: x-stationary, fp32r, fused taps).

out[b, o, l] = sum_{c,k} x[b, c, l+k] * weight[o, c, l, k] + bias[o, l]
  x: (32, 128, 2050) f32, weight: (128, 128, 2048, 3) f32, bias: (128, 2048) f32
  out: (32, 128, 2048) f32

Sharding: sequence-parallel over L across 8 cores (each core owns 256 output
positions, its private 50.3 MB weight slice, a 258-wide x window, and a
transposed bias slice).  Weight streaming from HBM is the roofline.

Per-core compute: out.T[b, l, o] = sum_{c} x[b, c, m] * W[o, c, l, m-l] for
each x column m.  The x column is the PE stationary operand (lhsT, K=128 c,
M=32 b, loaded once per column); the weights are the moving operand.  For one
column m, the contributions to l = m-2..m form an anti-diagonal in the
weight tile's (l, k) plane — stride 2 in the flattened l*3+k axis — so all
three taps fuse into ONE matmul with N = 3*128 = 384 moving columns.  With
float32r (single-pass fp32) and N >= 256 the PE streams 1 row/cycle.

PSUM: one bank holds out.T slice (32 b, 4 l, 128 o).  Each bank takes 7
matmuls: a K=1 ones x biasT matmul (start=True: clears the bank, seeds the
bias, sets every has_written bit) then 6 weight matmuls (m = 4j .. 4j+5,
clipped to the bank; per-element has_written makes them pure accumulates).
DVE then copies the bank to an (o, l)-ordered staging tile for the out DMA.
"""

import numpy as np

import concourse.bass as bass
import concourse.mybir as mybir
import concourse.tile as tile
from concourse.vector_clock import ScopedClock, VectorClock
from concourse.bass_utils import run_bass_kernel_spmd

# ---------------------------------------------------------------------------
# Environment patches
# ---------------------------------------------------------------------------

# The walrus build in this image rejects instructions with >1 sem wait; the
# Tile tail drain carries one wait per logical processor.  Split them into
# single-wait nops on SP before the drain.
def _patched_drain_and_barrier(self, tick_clock, wait_clock):
    gc = tick_clock.global_clock
    n = len(gc)
    for proc in range(n):
        t = gc[proc]
        if t <= 0:
            continue
        single = VectorClock([0] * n)
        single.require_at_least(proc, t)
        inst = self.nc.sync.nop(hint="tail_drain_wait")
        wait_clock.add_sem_waits(inst.ins, ScopedClock({None: single}))
    self.nc.sync.drain()
    self.nc.all_engine_barrier()
    assert self.sems is not None
    popped = self.nc._tile_sem_poison_stack.pop()
    assert popped is self._sem_poison
    self.nc.clear_and_free_semaphores(list(self.sems.allocated().values()))
    self.nc.all_engine_barrier()


if not getattr(tile.TileContext, "_drain_patch_applied", False):
    tile.TileContext._drain_and_barrier = _patched_drain_and_barrier
    tile.TileContext._drain_patch_applied = True


def _split_multi_waits(nc: bass.Bass) -> int:
    """Hoist all but the last wait of any multi-wait instruction onto
    single-wait nops inserted just before it in its engine's program order
    (the hardware takes one sem wait per instruction; this walrus build
    rejects multi-wait instructions instead of splitting them)."""
    n_split = 0
    for f in nc.m.functions:
        for bb in f.blocks:
            insts = list(bb.instructions)
            out = []
            for inst in insts:
                si = inst.sync_info
                if si is not None and len(si.on_wait) > 1:
                    waits = list(si.on_wait)
                    for w in waits[:-1]:
                        nop = mybir.InstNoOp(
                            name=nc.get_next_instruction_name(),
                            engine=inst.engine,
                            ins=[],
                            outs=[],
                            sync_info=mybir.SyncInfo(on_wait=[w], on_update=[]),
                        )
                        out.append(nop)
                    si.on_wait = [waits[-1]]
                    n_split += 1
                out.append(inst)
            bb.instructions = out
    return n_split

# ---------------------------------------------------------------------------
# Problem constants (hardcoded from the module spec)
# ---------------------------------------------------------------------------
N_CORES = 8
B = 32
CIN = 128
COUT = 128
L = 2048
KS = 3
W_FULL = 2050

LSH = L // N_CORES          # 256 output positions per core
WW = LSH + KS - 1           # 258-wide x window per core

LT = 32                     # l positions per weight tile / staging window
NWIN = LSH // LT            # 8 windows per core
BANKL = 4                   # l positions per PSUM bank (4*128 = 512 fp32)
NBANK = LT // BANKL         # 8 banks per window
BCH = 16                    # l rows per biasT SBUF chunk

F32 = mybir.dt.float32
F32R = mybir.dt.float32r


def _build_nc(split: bool = True) -> bass.Bass:
    nc = bass.Bass()

    x_d = nc.declare_dram_parameter("x", [B, CIN, WW], F32R, isOutput=False)
    w_d = nc.declare_dram_parameter("w", [COUT, CIN, LSH, KS], F32R, isOutput=False)
    bt_d = nc.declare_dram_parameter("biasT", [LSH, COUT], F32R, isOutput=False)
    ones_d = nc.declare_dram_parameter("ones", [1, B], F32R, isOutput=False)
    # (b, l, o) layout: staging DMAs out as 16 KB contiguous runs; the host
    # transposes back after gather.
    out_d = nc.declare_dram_parameter("out", [B, LSH, COUT], F32, isOutput=True)

    with tile.TileContext(nc) as tc:
        with (
            tc.tile_pool(name="xp", bufs=1) as xp,
            tc.tile_pool(name="cp", bufs=1) as cp,
            tc.tile_pool(name="wp", bufs=2) as wp,
            tc.tile_pool(name="bp", bufs=3) as bp,
            tc.tile_pool(name="sp", bufs=2) as sp,
            tc.tile_pool(name="pp", bufs=8, space="PSUM") as pp,
        ):
            # Persistent x in (c, b, w) layout: the stationary operand for
            # column m is x_sb[:, :, m] (K=128 c, M=32 b).  DMA runs 1032 B.
            x_sb = xp.tile([CIN, B, WW], F32R)
            # split so window 0's matmuls only wait on the first 64 columns
            nc.sync.dma_start(x_sb[:, :, 0:64],
                              x_d[:, :, 0:64].rearrange("b c w -> c b w"))
            nc.sync.dma_start(x_sb[:, :, 64:WW],
                              x_d[:, :, 64:WW].rearrange("b c w -> c b w"))

            ones = cp.tile([1, B], F32R)
            nc.sync.dma_start(ones[:], ones_d[:])

            for lc in range(NWIN):
                # weight tile (c, o, l*k); DRAM runs LT*KS*4 = 384 B
                w_t = wp.tile([CIN, COUT, LT * KS], F32R, tag="w", name="w_t")
                nc.sync.dma_start(
                    w_t[:],
                    w_d[:, :, lc * LT:(lc + 1) * LT, :]
                    .rearrange("o c l k -> c o (l k)"),
                )

                # biasT rows for this window, flattened on partition 0
                bias_ts = []
                for h in range(LT // BCH):
                    btile = bp.tile([1, BCH * COUT], F32R, tag="bt",
                                    name=f"bt_{lc}_{h}")
                    l0 = lc * LT + h * BCH
                    nc.sync.dma_start(
                        btile[:],
                        bt_d[l0:l0 + BCH, :].rearrange("l o -> (l o)")[None, :],
                    )
                    bias_ts.append(btile)

                st = sp.tile([B, LT, COUT], F32, tag="st", name=f"st_{lc}")

                for jb in range(NBANK):
                    ps = pp.tile([B, BANKL, COUT], F32, tag="ps", name="ps")
                    lw0 = jb * BANKL              # window-local l of bank start

                    # bias init: out[b, (l, o)] = 1[b] * biasT[(l, o)];
                    # start=True clears the bank and sets has_written.
                    bchunk = bias_ts[lw0 // BCH]
                    boff = (lw0 % BCH) * COUT
                    nc.tensor.matmul(
                        ps[:].rearrange("b l o -> b (l o)"),
                        ones[:],
                        bchunk[0:1, boff:boff + BANKL * COUT],
                        start=True,
                        stop=False,
                        skip_group_check=True,
                    )

                    # six weight matmuls: x columns m = bank start .. +5
                    for d in range(BANKL + KS - 1):
                        mw = lw0 + d                  # window-local x column
                        m = lc * LT + mw              # shard-local x column
                        lo = max(lw0, mw - (KS - 1))  # window-local l' range
                        hi = min(lw0 + BANKL - 1, mw)
                        nl = hi - lo + 1
                        # anti-diagonal AP over the weight tile: element
                        # (o, l', k=mw-l') at o*(LT*KS) + l'*3 + (mw-l')
                        # -> l' step 2, o step LT*KS
                        rhs = bass.AP(
                            w_t[:].tensor,
                            lo * KS + (mw - lo),
                            [[COUT * LT * KS, CIN], [2, nl], [LT * KS, COUT]],
                        )
                        nc.tensor.matmul(
                            ps[:, lo - lw0:hi - lw0 + 1, :],
                            x_sb[:, :, m],
                            rhs,
                            start=False,
                            stop=(d == BANKL + KS - 2),
                            skip_group_check=True,
                        )

                    # PSUM (b, l, o) -> staging (b, l, o), plain copy
                    nc.vector.tensor_copy(
                        st[:, lw0:lw0 + BANKL, :],
                        ps[:],
                    )

                nc.scalar.dma_start(out_d[:, lc * LT:(lc + 1) * LT, :], st[:])

    if split:
        _split_multi_waits(nc)
    return nc


_NC_CACHE = None


def _get_nc() -> bass.Bass:
    global _NC_CACHE
    if _NC_CACHE is None:
        _NC_CACHE = _build_nc()
    return _NC_CACHE


def shard_inputs(x, weight, bias):
    x = np.asarray(x, dtype=np.float32)
    weight = np.asarray(weight, dtype=np.float32)
    bias = np.asarray(bias, dtype=np.float32)
    in_maps = []
    for i in range(N_CORES):
        l0 = i * LSH
        in_maps.append({
            "x": np.ascontiguousarray(x[:, :, l0:l0 + WW]),
            "w": np.ascontiguousarray(weight[:, :, l0:l0 + LSH, :]),
            "biasT": np.ascontiguousarray(bias[:, l0:l0 + LSH].T),
            "ones": np.ones((1, B), dtype=np.float32),
        })
    return in_maps


def gather_output(results):
    out = np.empty((B, COUT, L), dtype=np.float32)
    for i in range(N_CORES):
        out[:, :, i * LSH:(i + 1) * LSH] = results[i]["out"].transpose(0, 2, 1)
    return out


def kernel(x, weight, bias):
    nc = _get_nc()
    in_maps = shard_inputs(x, weight, bias)
    res = run_bass_kernel_spmd(nc, in_maps, core_ids=list(range(N_CORES)),
                               trace=False)
    return gather_output(res.results)


# revision 12
# speedup vs baseline: 2.3514x; 1.3256x over previous
"""LocallyConnected1d Trainium2 kernel (v4: x-stationary fp32r matmuls,
fused kernel taps, host-pretiled weights).

out[b, o, l] = sum_{c,k} x[b, c, l+k] * weight[o, c, l, k] + bias[o, l]
  x: (32, 128, 2050) f32, weight: (128, 128, 2048, 3) f32, bias: (128, 2048) f32
  out: (32, 128, 2048) f32

Sharding: sequence-parallel over L across 8 cores (each core owns 256 output
positions, its private 50.3 MB weight slice, a 258-wide x window, and a
transposed bias slice).  Weight streaming from HBM is the roofline; the host
lays each core's weight shard out as the exact per-window SBUF tile images
(c, o, l, k) so every weight DMA descriptor is one contiguous 24 KB run.

Per-core compute: out.T[b, l, o] = sum_c x[b, c, m] * W[o, c, l, m-l] per x
column m.  The x column is the PE stationary operand (K=128 c, M=32 b); the
weights are the moving operand.  For one column m the contributions to
l = m-2..m form an anti-diagonal of the weight tile's (l, k) plane — stride 2
in the flattened l*3+k axis — so all three taps fuse into ONE matmul with
N = 3*128 = 384 moving columns.  float32r (single-pass fp32) streams
1 row/cycle at N >= 256.

PSUM: one bank holds out.T slice (32 b, 4 l, 128 o).  Each bank takes 7
matmuls: a K=1 ones x biasT matmul (start=True clears the bank, seeds the
bias, sets every has_written bit), then 6 weight matmuls (m = 4j..4j+5
clipped to the bank; per-element has_written makes them pure accumulates).
DVE copies each bank to (b, l, o)-ordered staging; the out DMA writes
contiguous runs and the host transposes after gather.
"""

import numpy as np

import concourse.bass as bass
import concourse.mybir as mybir
import concourse.tile as tile
from concourse.vector_clock import ScopedClock, VectorClock
from concourse.bass_utils import run_bass_kernel_spmd

# ---------------------------------------------------------------------------
# Environment patches
# ---------------------------------------------------------------------------

# The walrus build in this image rejects instructions with >1 sem wait; the
# Tile tail drain carries one wait per logical processor.  Split them into
# single-wait nops on SP before the drain.
def _patched_drain_and_barrier(self, tick_clock, wait_clock):
    gc = tick_clock.global_clock
    n = len(gc)
    for proc in range(n):
        t = gc[proc]
        if t <= 0:
            continue
        single = VectorClock([0] * n)
        single.require_at_least(proc, t)
        inst = self.nc.sync.nop(hint="tail_drain_wait")
        wait_clock.add_sem_waits(inst.ins, ScopedClock({None: single}))
    self.nc.sync.drain()
    self.nc.all_engine_barrier()
    assert self.sems is not None
    popped = self.nc._tile_sem_poison_stack.pop()
    assert popped is self._sem_poison
    self.nc.clear_and_free_semaphores(list(self.sems.allocated().values()))
    self.nc.all_engine_barrier()


if not getattr(tile.TileContext, "_drain_patch_applied", False):
    tile.TileContext._drain_and_barrier = _patched_drain_and_barrier
    tile.TileContext._drain_patch_applied = True


def _split_multi_waits(nc: bass.Bass) -> int:
    """Hoist all but the last wait of any multi-wait instruction onto
    single-wait nops inserted just before it in its engine's program order
    (the hardware takes one sem wait per instruction; this walrus build
    rejects multi-wait instructions instead of splitting them)."""
    n_split = 0
    for f in nc.m.functions:
        for bb in f.blocks:
            insts = list(bb.instructions)
            out = []
            for inst in insts:
                si = inst.sync_info
                if si is not None and len(si.on_wait) > 1:
                    waits = list(si.on_wait)
                    for w in waits[:-1]:
                        nop = mybir.InstNoOp(
                            name=nc.get_next_instruction_name(),
                            engine=inst.engine,
                            ins=[],
                            outs=[],
                            sync_info=mybir.SyncInfo(on_wait=[w], on_update=[]),
                        )
                        out.append(nop)
                    si.on_wait = [waits[-1]]
                    n_split += 1
                out.append(inst)
            bb.instructions = out
    return n_split

# ---------------------------------------------------------------------------
# Problem constants (hardcoded from the module spec)
# ---------------------------------------------------------------------------
N_CORES = 8
B = 32
CIN = 128
COUT = 128
L = 2048
KS = 3
W_FULL = 2050

LSH = L // N_CORES          # 256 output positions per core
WW = LSH + KS - 1           # 258-wide x window per core

LT = 16                     # l positions per weight tile / staging window
NWIN = LSH // LT            # 16 windows per core
BANKL = 4                   # l positions per PSUM bank (4*128 = 512 fp32)
NBANK = LT // BANKL         # 4 banks per window
WFREE = COUT * LT * KS      # weight tile free size (6144 fp32 = 24 KB)

F32 = mybir.dt.float32
F32R = mybir.dt.float32r


def _build_nc(split: bool = True) -> bass.Bass:
    nc = bass.Bass()

    x_d = nc.declare_dram_parameter("x", [B, CIN, WW], F32R, isOutput=False)
    wt_d = nc.declare_dram_parameter("wt", [NWIN, CIN, WFREE], F32R,
                                     isOutput=False)
    bt_d = nc.declare_dram_parameter("biasT", [LSH, COUT], F32R, isOutput=False)
    ones_d = nc.declare_dram_parameter("ones", [1, B], F32R, isOutput=False)
    # (b, l, o) layout: staging DMAs out as contiguous runs; the host
    # transposes back after gather.
    out_d = nc.declare_dram_parameter("out", [B, LSH, COUT], F32, isOutput=True)

    with tile.TileContext(nc) as tc:
        with (
            tc.tile_pool(name="xp", bufs=1) as xp,
            tc.tile_pool(name="cp", bufs=1) as cp,
            tc.tile_pool(name="wp", bufs=4) as wp,
            tc.tile_pool(name="bp", bufs=3) as bp,
            tc.tile_pool(name="sp", bufs=3) as sp,
            tc.tile_pool(name="pp", bufs=8, space="PSUM") as pp,
        ):
            # Persistent x in (c, b, w) layout: the stationary operand for
            # column m is x_sb[:, :, m] (K=128 c, M=32 b).  DMA runs 1032 B.
            x_sb = xp.tile([CIN, B, WW], F32R)
            # split so window 0's matmuls only wait on the first columns
            nc.sync.dma_start(x_sb[:, :, 0:32],
                              x_d[:, :, 0:32].rearrange("b c w -> c b w"))
            nc.sync.dma_start(x_sb[:, :, 32:WW],
                              x_d[:, :, 32:WW].rearrange("b c w -> c b w"))

            ones = cp.tile([1, B], F32R)
            nc.sync.dma_start(ones[:], ones_d[:])

            for lc in range(NWIN):
                # weight tile (c, o, l*k); one contiguous 24 KB run/partition
                w_t = wp.tile([CIN, COUT, LT * KS], F32R, tag="w", name="w_t")
                nc.sync.dma_start(
                    w_t[:].rearrange("c o f -> c (o f)"), wt_d[lc]
                )

                # biasT rows for this window, flattened on partition 0
                btile = bp.tile([1, LT * COUT], F32R, tag="bt",
                                name=f"bt_{lc}")
                nc.sync.dma_start(
                    btile[:],
                    bt_d[lc * LT:(lc + 1) * LT, :]
                    .rearrange("l o -> (l o)")[None, :],
                )

                st = sp.tile([B, LT, COUT], F32, tag="st", name=f"st_{lc}")

                for jb in range(NBANK):
                    ps = pp.tile([B, BANKL, COUT], F32, tag="ps", name="ps")
                    lw0 = jb * BANKL              # window-local l of bank start

                    # bias init: out[b, (l, o)] = 1[b] * biasT[(l, o)];
                    # start=True clears the bank and sets has_written.
                    boff = lw0 * COUT
                    nc.tensor.matmul(
                        ps[:].rearrange("b l o -> b (l o)"),
                        ones[:],
                        btile[0:1, boff:boff + BANKL * COUT],
                        start=True,
                        stop=False,
                        skip_group_check=True,
                    )

                    # six weight matmuls: x columns m = bank start .. +5
                    for d in range(BANKL + KS - 1):
                        mw = lw0 + d                  # window-local x column
                        m = lc * LT + mw              # shard-local x column
                        lo = max(lw0, mw - (KS - 1))  # window-local l' range
                        hi = min(lw0 + BANKL - 1, mw)
                        nl = hi - lo + 1
                        # anti-diagonal AP over the weight tile: element
                        # (o, l', k=mw-l') at o*(LT*KS) + l'*3 + (mw-l')
                        # -> l' step 2, o step LT*KS
                        rhs = bass.AP(
                            w_t[:].tensor,
                            lo * KS + (mw - lo),
                            [[COUT * LT * KS, CIN], [2, nl], [LT * KS, COUT]],
                        )
                        nc.tensor.matmul(
                            ps[:, lo - lw0:hi - lw0 + 1, :],
                            x_sb[:, :, m],
                            rhs,
                            start=False,
                            stop=(d == BANKL + KS - 2),
                            skip_group_check=True,
                        )

                    # PSUM (b, l, o) -> staging (b, l, o), plain copy
                    nc.vector.tensor_copy(
                        st[:, lw0:lw0 + BANKL, :],
                        ps[:],
                    )

                nc.scalar.dma_start(out_d[:, lc * LT:(lc + 1) * LT, :], st[:])

    if split:
        _split_multi_waits(nc)
    return nc


_NC_CACHE = None


def _get_nc() -> bass.Bass:
    global _NC_CACHE
    if _NC_CACHE is None:
        _NC_CACHE = _build_nc()
    return _NC_CACHE


def _tile_weights(w_shard: np.ndarray) -> np.ndarray:
    """(COUT, CIN, LSH, KS) -> (NWIN, CIN, COUT*LT*KS) per-window SBUF tile
    images: wt[n, c, o*LT*KS + l*KS + k] = w_shard[o, c, n*LT + l, k]."""
    w = w_shard.transpose(1, 0, 2, 3)                  # (CIN, COUT, LSH, KS)
    w = w.reshape(CIN, COUT, NWIN, LT, KS)
    w = w.transpose(2, 0, 1, 3, 4)                     # (NWIN, CIN, COUT, LT, KS)
    return np.ascontiguousarray(w.reshape(NWIN, CIN, WFREE))


def shard_inputs(x, weight, bias):
    x = np.asarray(x, dtype=np.float32)
    weight = np.asarray(weight, dtype=np.float32)
    bias = np.asarray(bias, dtype=np.float32)
    in_maps = []
    for i in range(N_CORES):
        l0 = i * LSH
        in_maps.append({
            "x": np.ascontiguousarray(x[:, :, l0:l0 + WW]),
            "wt": _tile_weights(weight[:, :, l0:l0 + LSH, :]),
            "biasT": np.ascontiguousarray(bias[:, l0:l0 + LSH].T),
            "ones": np.ones((1, B), dtype=np.float32),
        })
    return in_maps


def gather_output(results):
    out = np.empty((B, COUT, L), dtype=np.float32)
    for i in range(N_CORES):
        out[:, :, i * LSH:(i + 1) * LSH] = results[i]["out"].transpose(0, 2, 1)
    return out


def kernel(x, weight, bias):
    nc = _get_nc()
    in_maps = shard_inputs(x, weight, bias)
    res = run_bass_kernel_spmd(nc, in_maps, core_ids=list(range(N_CORES)),
                               trace=False)
    return gather_output(res.results)
